# revision 27
# baseline (speedup 1.0000x reference)
"""Block-diagonal rotation (COB) kernel for Trainium2, 8 NeuronCores.

Computes out[..., block_i] = x[..., block_i] @ W_i.T for 8 square blocks of
sizes [512, 1024, 256, 768, 384, 640, 128, 384] (features sum to 4096),
x shape (4, 2048, 4096) fp32.

Strategy (bf16 everywhere, host-side layout transforms):
  - Pure data-parallel over rows: 8192 rows split 8 ways (1024 rows/core).
  - All tensors cast to bf16 on the host (rel-err budget is 2e-2; bf16
    end-to-end measures ~4e-3).  This halves HBM traffic vs fp32:
    21.1 MiB/core (x 8 + w 5.1 + out 8) vs 42.25 MiB.
  - x is transposed on the host, so the kernel works on x^T stripes
    [128 d, 1024 r] with zero PE transposes (the v1 kernel spent ~40 us
    of PE time transposing x on-chip).
  - Host pre-packs x^T and W^T into chunk-major, partition-major blobs
    ([128, cn*1024] covering up to 4 k-stripes), so every in-DMA is one
    plain fully-sequential [p, f] transfer; the whole in-stream is ~22
    DMA instructions.  Fewer DMAs = less DMAHW-lane (8 global lanes)
    reuse-serialization between the in and out streams, which otherwise
    delays out-DMA issue by multiple us.
  - Stationary operand = x^T 128x128 sub-tile, moving operand = resident
    W^T stripe slice [128, nw<=512]; psum accumulates over each block's
    contraction k-tiles; out copied psum->SBUF as bf16 (alternating
    ACT/DVE) into a per-block [128, 8*s_b] staging tile, then ONE
    sequential out-DMA per block (p-major DRAM layout, host unpacks).
  - Loop order: block-outer, e-slice, k-middle, row-tile-inner.  With
    k-middle, each incoming stripe unlocks 8 row-tiles of matmuls, so
    the PE streams right behind the in-DMA with no phase stalls.
    Weights + out ride the ACT HWDGE ring, x rides the SP ring.
  - ~10 warm-up matmuls on a zeroed tile during the DMA prologue keep
    the HAM clock gate at K=8/8 (2.4 GHz) when real matmuls start.

Roofline: PE-bound — 5.5 GFLOP/core / 78.6 TFLOP/s bf16 = 70 us matmul
streaming; DMA 21.1 MiB / ~358 GB/s = 59 us hides under it.
"""

import numpy as np
import ml_dtypes

import concourse.bacc as bacc
import concourse.mybir as mybir
from concourse.tile import TileContext
from concourse.bass_utils import run_bass_kernel_spmd

SIZES = [512, 1024, 256, 768, 384, 640, 128, 384]
OFFS = np.cumsum([0] + SIZES)
N_CORES = 8
ROWS_TOTAL = 4 * 2048
RPC = ROWS_TOTAL // N_CORES  # 1024 rows per core
D = 4096
P = 128
R_TILES = RPC // P  # 8
G_TILES = D // P  # 32 x^T stripes

BF16 = mybir.dt.bfloat16
F32 = mybir.dt.float32

# psum slices per block (<=512 = one PSUM bank of fp32); larger slice first
# so the stripe-gated first pass of each block has the most PE work per stripe
E_SLICES = {
    512: [512], 1024: [512, 512], 256: [256], 768: [512, 256],
    384: [384], 640: [384, 256], 128: [128],
}

# medium block first (good compute-per-byte while the pipe fills), big blocks
# early to build DMA slack, small blocks interleaved mid-stream so their
# out-DMAs don't bunch into the drain, tiny block last
BLOCK_ORDER = [0, 1, 4, 3, 7, 5, 2, 6]

CHUNK = 2  # max k-stripes per merged in-DMA (0.5 MB x chunks: big enough to
           # amortize DMA fixed cost, small enough to recycle the ~8 global
           # DMAHW lanes quickly and keep PE data-arrival granularity fine)


def chunks_of(b):
    kt = SIZES[b] // P
    return [(c0, min(CHUNK, kt - c0)) for c0 in range(0, kt, CHUNK)]


# flat offsets of each x chunk in the per-core packed x array (natural order)
X_OFF = {}
_off = 0
for _b in range(len(SIZES)):
    for _c0, _cn in chunks_of(_b):
        X_OFF[(_b, _c0)] = _off
        _off += P * _cn * RPC
assert _off == D * RPC

_cache = {}


def build_nc():
    if "nc" in _cache:
        return _cache["nc"]
    nc = bacc.Bacc()
    xt_d = nc.declare_dram_parameter("xt", [D * RPC], BF16, isOutput=False)
    w_d = [
        nc.declare_dram_parameter(f"w{i}", [s * s], BF16, isOutput=False)
        for i, s in enumerate(SIZES)
    ]
    # out is block-major and partition-major: for each block b, a
    # [128 p, 8 r, s_b] chunk at element offset OFFS[b]*RPC; host unpacks
    out_d = nc.declare_dram_parameter("out", [RPC * D], BF16, isOutput=True)

    with TileContext(nc) as tc:
        with (
            tc.tile_pool(name="xres", bufs=1) as xres,
            tc.tile_pool(name="wres", bufs=1) as wres,
            tc.tile_pool(name="osb", bufs=1) as osb_p,
            tc.tile_pool(name="wu", bufs=1) as wu_p,
            tc.tile_pool(name="mm", bufs=8, space="PSUM") as mm_p,
        ):
            # PE warm-up: dummy matmuls on a zeroed tile fill the otherwise
            # idle DMA prologue with PE activity so the HAM clock gate is at
            # K=8/8 when the first real matmul lands.
            wu_sb = wu_p.tile([P, 512], BF16, tag="wusb", name="wu_sb")
            nc.vector.memset(wu_sb[:], 0.0)
            wu_ps = mm_p.tile([P, 512], F32, tag="mmps", name="wu_ps")
            for _ in range(10):
                nc.tensor.matmul(wu_ps[:], wu_sb[:, :P], wu_sb[:],
                                 start=True, stop=True)

            first = True
            for bi, b in enumerate(BLOCK_ORDER):
                s = SIZES[b]
                kt = s // P

                # merged chunk loads, all plain sequential [p, f] transfers,
                # all on the SP ring in exact consumption order (w chunk then
                # x chunk per k-range); the ACT ring carries ONLY out-DMAs so
                # they never queue behind in-DMA lane-waits
                wk, xs = [], []
                for c0, cn in chunks_of(b):
                    wt = wres.tile([P, cn * s], BF16, tag=f"w{b}_{c0}",
                                   name="wt")
                    w_src = w_d[b][c0 * P * s:(c0 + cn) * P * s].rearrange(
                        "(p f) -> p f", p=P)
                    nc.sync.dma_start(out=wt[:], in_=w_src)
                    wk += [(wt, k * s) for k in range(cn)]

                    xk = xres.tile([P, cn * RPC], BF16, tag=f"x{b}_{c0}",
                                   name="xk")
                    xo = X_OFF[(b, c0)]
                    x_src = xt_d[xo:xo + P * cn * RPC].rearrange(
                        "(p f) -> p f", p=P)
                    if first and c0 == 0:
                        # prologue: first stripe in 4 column-chunks, then
                        # per-stripe, so the first matmuls start early
                        q = RPC // 4
                        for c in range(4):
                            nc.sync.dma_start(out=xk[:, c * q:(c + 1) * q],
                                              in_=x_src[:, c * q:(c + 1) * q])
                        for k in range(1, cn):
                            nc.sync.dma_start(
                                out=xk[:, k * RPC:(k + 1) * RPC],
                                in_=x_src[:, k * RPC:(k + 1) * RPC])
                    else:
                        nc.sync.dma_start(out=xk[:], in_=x_src)
                    xs += [(xk, k * RPC) for k in range(cn)]
                first = False

                # one staging tile per block: [128 p, (8 r) * s_b] bf16
                ot = osb_p.tile([P, R_TILES * s], BF16, tag=f"o{b}", name="ot")
                n0 = 0
                for si, nw in enumerate(E_SLICES[s]):
                    pss = [
                        mm_p.tile([P, 512], F32, tag="mmps", name="ps")
                        for _ in range(R_TILES)
                    ]
                    for k in range(kt):
                        xt_t, xb = xs[k]
                        wt_t, wb = wk[k]
                        for r in range(R_TILES):
                            nc.tensor.matmul(
                                pss[r][:, :nw],
                                xt_t[:, xb + P * r:xb + P * (r + 1)],
                                wt_t[:, wb + n0:wb + n0 + nw],
                                start=(k == 0), stop=(k == kt - 1),
                            )
                    # psum -> bf16 staging, alternating ACT/DVE
                    for r in range(R_TILES):
                        dst = ot[:, r * s + n0:r * s + n0 + nw]
                        if (r + si) % 2 == 0:
                            nc.scalar.copy(dst, pss[r][:, :nw])
                        else:
                            nc.vector.tensor_copy(dst, pss[r][:, :nw])
                    n0 += nw
                # one sequential out-DMA per block: SBUF [p, (r e)] -> DRAM
                # [p, (r e)] chunk.  The last blocks' outs ride the (by then
                # idle) SP ring so the drain doesn't serialize behind earlier
                # outs on the ACT ring's FIFO.
                out_b = out_d[int(OFFS[b]) * RPC:int(OFFS[b + 1]) * RPC]
                out_v = out_b.rearrange("(p f) -> p f", p=P)
                oeng = nc.sync if bi >= 5 else nc.scalar
                oeng.dma_start(out=out_v, in_=ot[:])

    nc.finalize()
    _cache["nc"] = nc
    return nc


def pack_chunks(stripes, cn_list):
    """stripes: [n_stripes, P, W] -> flat concat of p-major chunk blobs."""
    blobs = []
    g = 0
    for cn in cn_list:
        blob = stripes[g:g + cn].transpose(1, 0, 2).reshape(P, cn * stripes.shape[2])
        blobs.append(blob.reshape(-1))
        g += cn
    return np.concatenate(blobs)


def build_in_maps(x, w0, w1, w2, w3, w4, w5, w6, w7):
    bf = ml_dtypes.bfloat16
    x = np.asarray(x, dtype=np.float32).reshape(ROWS_TOTAL, D)
    xt = np.ascontiguousarray(x.astype(bf).T)  # [D, ROWS_TOTAL]
    ws = [w0, w1, w2, w3, w4, w5, w6, w7]

    all_cn = [cn for b in range(len(SIZES)) for _, cn in chunks_of(b)]
    w_packed = []
    for i, w in enumerate(ws):
        wts = np.asarray(w, dtype=np.float32).T.astype(bf)  # W^T [s, s]
        s = SIZES[i]
        stripes = wts.reshape(s // P, P, s)
        w_packed.append(pack_chunks(stripes, [cn for _, cn in chunks_of(i)]))

    in_maps = []
    for c in range(N_CORES):
        xc = np.ascontiguousarray(xt[:, c * RPC:(c + 1) * RPC])
        stripes = xc.reshape(G_TILES, P, RPC)
        m = {"xt": pack_chunks(stripes, all_cn)}
        for i, wp in enumerate(w_packed):
            m[f"w{i}"] = wp
        in_maps.append(m)
    return in_maps


def unshard_out(results):
    out = np.empty((ROWS_TOTAL, D), dtype=np.float32)
    for c, r in enumerate(results):
        buf = np.asarray(r["out"])  # flat [RPC * D] bf16, block+partition-major
        rows = slice(c * RPC, (c + 1) * RPC)
        for b, s in enumerate(SIZES):
            seg = buf[int(OFFS[b]) * RPC:int(OFFS[b + 1]) * RPC]
            # p-major: [p, r, e] -> rows r*128+p
            seg = seg.reshape(P, R_TILES, s).transpose(1, 0, 2).reshape(RPC, s)
            out[rows, int(OFFS[b]):int(OFFS[b + 1])] = seg
    return out


def kernel(x, w0, w1, w2, w3, w4, w5, w6, w7):
    nc = build_nc()
    in_maps = build_in_maps(x, w0, w1, w2, w3, w4, w5, w6, w7)
    res = run_bass_kernel_spmd(nc, in_maps, list(range(N_CORES)))
    return unshard_out(res.results).reshape(4, 2048, D)


# revision 28
# speedup vs baseline: 1.1701x; 1.1701x over previous
"""Block-diagonal rotation (COB) kernel for Trainium2, 8 NeuronCores.

Computes out[..., block_i] = x[..., block_i] @ W_i.T for 8 square blocks of
sizes [512, 1024, 256, 768, 384, 640, 128, 384] (features sum to 4096),
x shape (4, 2048, 4096) fp32.

Strategy (bf16 everywhere, host-side layout transforms):
  - Pure data-parallel over rows: 8192 rows split 8 ways (1024 rows/core).
  - All tensors cast to bf16 on the host (rel-err budget is 2e-2; bf16
    end-to-end measures ~4e-3).  This halves HBM traffic vs fp32:
    21.1 MiB/core (x 8 + w 5.1 + out 8) vs 42.25 MiB.
  - x is transposed on the host, so the kernel works on x^T stripes
    [128 d, 1024 r] with zero PE transposes (the v1 kernel spent ~40 us
    of PE time transposing x on-chip).
  - Host pre-packs x^T and W^T into chunk-major, partition-major blobs
    ([128, cn*1024] covering up to 4 k-stripes), so every in-DMA is one
    plain fully-sequential [p, f] transfer; the whole in-stream is ~22
    DMA instructions.  Fewer DMAs = less DMAHW-lane (8 global lanes)
    reuse-serialization between the in and out streams, which otherwise
    delays out-DMA issue by multiple us.
  - Stationary operand = x^T 128x128 sub-tile, moving operand = resident
    W^T stripe slice [128, nw<=512]; psum accumulates over each block's
    contraction k-tiles; out copied psum->SBUF as bf16 (alternating
    ACT/DVE) into a per-block [128, 8*s_b] staging tile, then ONE
    sequential out-DMA per block (p-major DRAM layout, host unpacks).
  - Loop order: block-outer, e-slice, k-middle, row-tile-inner.  With
    k-middle, each incoming stripe unlocks 8 row-tiles of matmuls, so
    the PE streams right behind the in-DMA with no phase stalls.
    Weights + out ride the ACT HWDGE ring, x rides the SP ring.
  - ~10 warm-up matmuls on a zeroed tile during the DMA prologue keep
    the HAM clock gate at K=8/8 (2.4 GHz) when real matmuls start.

Roofline: PE-bound — 5.5 GFLOP/core / 78.6 TFLOP/s bf16 = 70 us matmul
streaming; DMA 21.1 MiB / ~358 GB/s = 59 us hides under it.
"""

import numpy as np
import ml_dtypes

import concourse.bacc as bacc
import concourse.mybir as mybir
from concourse.tile import TileContext
from concourse.bass_utils import run_bass_kernel_spmd

SIZES = [512, 1024, 256, 768, 384, 640, 128, 384]
OFFS = np.cumsum([0] + SIZES)
N_CORES = 8
ROWS_TOTAL = 4 * 2048
RPC = ROWS_TOTAL // N_CORES  # 1024 rows per core
D = 4096
P = 128
R_TILES = RPC // P  # 8
G_TILES = D // P  # 32 x^T stripes

BF16 = mybir.dt.bfloat16
F32 = mybir.dt.float32

# psum slices per block (<=512 = one PSUM bank of fp32); larger slice first
# so the stripe-gated first pass of each block has the most PE work per stripe
E_SLICES = {
    512: [512], 1024: [512, 512], 256: [256], 768: [512, 256],
    384: [384], 640: [384, 256], 128: [128],
}

# medium block first (good compute-per-byte while the pipe fills), big blocks
# early to build DMA slack, small blocks interleaved mid-stream so their
# out-DMAs don't bunch into the drain, tiny block last
BLOCK_ORDER = [0, 1, 4, 3, 7, 5, 2, 6]

CHUNK = 2  # max k-stripes per merged in-DMA (0.5 MB x chunks: big enough to
           # amortize DMA fixed cost, small enough to recycle the ~8 global
           # DMAHW lanes quickly and keep PE data-arrival granularity fine)


def chunks_of(b):
    kt = SIZES[b] // P
    return [(c0, min(CHUNK, kt - c0)) for c0 in range(0, kt, CHUNK)]


# flat offsets of each x chunk in the per-core packed x array (natural order)
X_OFF = {}
_off = 0
for _b in range(len(SIZES)):
    for _c0, _cn in chunks_of(_b):
        X_OFF[(_b, _c0)] = _off
        _off += P * _cn * RPC
assert _off == D * RPC

_cache = {}


def build_nc():
    if "nc" in _cache:
        return _cache["nc"]
    nc = bacc.Bacc()
    xt_d = nc.declare_dram_parameter("xt", [D * RPC], BF16, isOutput=False)
    w_d = [
        nc.declare_dram_parameter(f"w{i}", [s * s], BF16, isOutput=False)
        for i, s in enumerate(SIZES)
    ]
    # out is block-major and partition-major: for each block b, a
    # [128 p, 8 r, s_b] chunk at element offset OFFS[b]*RPC; host unpacks
    out_d = nc.declare_dram_parameter("out", [RPC * D], BF16, isOutput=True)

    with TileContext(nc) as tc:
        with (
            tc.tile_pool(name="xres", bufs=1) as xres,
            tc.tile_pool(name="wres", bufs=1) as wres,
            tc.tile_pool(name="osb", bufs=1) as osb_p,
            tc.tile_pool(name="wu", bufs=1) as wu_p,
            tc.tile_pool(name="mm", bufs=8, space="PSUM") as mm_p,
        ):
            # PE warm-up: dummy matmuls on a zeroed tile fill the otherwise
            # idle DMA prologue with PE activity so the HAM clock gate is at
            # K=8/8 when the first real matmul lands.
            wu_sb = wu_p.tile([P, 512], BF16, tag="wusb", name="wu_sb")
            nc.vector.memset(wu_sb[:], 0.0)
            wu_ps = mm_p.tile([P, 512], F32, tag="mmps", name="wu_ps")
            for _ in range(10):
                nc.tensor.matmul(wu_ps[:], wu_sb[:, :P], wu_sb[:],
                                 start=True, stop=True)

            first = True
            for bi, b in enumerate(BLOCK_ORDER):
                s = SIZES[b]
                kt = s // P

                # merged chunk loads, all plain sequential [p, f] transfers,
                # all on the SP ring in exact consumption order (w chunk then
                # x chunk per k-range); the ACT ring carries ONLY out-DMAs so
                # they never queue behind in-DMA lane-waits
                wk, xs = [], []
                for c0, cn in chunks_of(b):
                    wt = wres.tile([P, cn * s], BF16, tag=f"w{b}_{c0}",
                                   name="wt")
                    w_src = w_d[b][c0 * P * s:(c0 + cn) * P * s].rearrange(
                        "(p f) -> p f", p=P)
                    nc.sync.dma_start(out=wt[:], in_=w_src)
                    wk += [(wt, k * s) for k in range(cn)]

                    xk = xres.tile([P, cn * RPC], BF16, tag=f"x{b}_{c0}",
                                   name="xk")
                    xo = X_OFF[(b, c0)]
                    x_src = xt_d[xo:xo + P * cn * RPC].rearrange(
                        "(p f) -> p f", p=P)
                    if first and c0 == 0:
                        # prologue: first stripe in 4 column-chunks, then
                        # per-stripe, so the first matmuls start early
                        q = RPC // 4
                        for c in range(4):
                            nc.sync.dma_start(out=xk[:, c * q:(c + 1) * q],
                                              in_=x_src[:, c * q:(c + 1) * q])
                        for k in range(1, cn):
                            nc.sync.dma_start(
                                out=xk[:, k * RPC:(k + 1) * RPC],
                                in_=x_src[:, k * RPC:(k + 1) * RPC])
                    else:
                        nc.sync.dma_start(out=xk[:], in_=x_src)
                    xs += [(xk, k * RPC) for k in range(cn)]
                first = False

                # one staging tile per block: [128 p, (8 r) * s_b] bf16
                ot = osb_p.tile([P, R_TILES * s], BF16, tag=f"o{b}", name="ot")
                n0 = 0
                for si, nw in enumerate(E_SLICES[s]):
                    pss = [
                        mm_p.tile([P, 512], F32, tag="mmps", name="ps")
                        for _ in range(R_TILES)
                    ]
                    for k in range(kt):
                        xt_t, xb = xs[k]
                        wt_t, wb = wk[k]
                        for r in range(R_TILES):
                            nc.tensor.matmul(
                                pss[r][:, :nw],
                                xt_t[:, xb + P * r:xb + P * (r + 1)],
                                wt_t[:, wb + n0:wb + n0 + nw],
                                start=(k == 0), stop=(k == kt - 1),
                            )
                    # psum -> bf16 staging, alternating ACT/DVE
                    for r in range(R_TILES):
                        dst = ot[:, r * s + n0:r * s + n0 + nw]
                        if (r + si) % 2 == 0:
                            nc.scalar.copy(dst, pss[r][:, :nw])
                        else:
                            nc.vector.tensor_copy(dst, pss[r][:, :nw])
                    n0 += nw
                # one sequential out-DMA per block: SBUF [p, (r e)] -> DRAM
                # [p, (r e)] chunk.  The last blocks' outs ride the (by then
                # idle) SP ring so the drain doesn't serialize behind earlier
                # outs on the ACT ring's FIFO.
                out_b = out_d[int(OFFS[b]) * RPC:int(OFFS[b + 1]) * RPC]
                out_v = out_b.rearrange("(p f) -> p f", p=P)
                oeng = nc.sync if bi == len(BLOCK_ORDER) - 1 else nc.scalar
                oeng.dma_start(out=out_v, in_=ot[:])

    nc.finalize()
    _cache["nc"] = nc
    return nc


def pack_chunks(stripes, cn_list):
    """stripes: [n_stripes, P, W] -> flat concat of p-major chunk blobs."""
    blobs = []
    g = 0
    for cn in cn_list:
        blob = stripes[g:g + cn].transpose(1, 0, 2).reshape(P, cn * stripes.shape[2])
        blobs.append(blob.reshape(-1))
        g += cn
    return np.concatenate(blobs)


def build_in_maps(x, w0, w1, w2, w3, w4, w5, w6, w7):
    bf = ml_dtypes.bfloat16
    x = np.asarray(x, dtype=np.float32).reshape(ROWS_TOTAL, D)
    xt = np.ascontiguousarray(x.astype(bf).T)  # [D, ROWS_TOTAL]
    ws = [w0, w1, w2, w3, w4, w5, w6, w7]

    all_cn = [cn for b in range(len(SIZES)) for _, cn in chunks_of(b)]
    w_packed = []
    for i, w in enumerate(ws):
        wts = np.asarray(w, dtype=np.float32).T.astype(bf)  # W^T [s, s]
        s = SIZES[i]
        stripes = wts.reshape(s // P, P, s)
        w_packed.append(pack_chunks(stripes, [cn for _, cn in chunks_of(i)]))

    in_maps = []
    for c in range(N_CORES):
        xc = np.ascontiguousarray(xt[:, c * RPC:(c + 1) * RPC])
        stripes = xc.reshape(G_TILES, P, RPC)
        m = {"xt": pack_chunks(stripes, all_cn)}
        for i, wp in enumerate(w_packed):
            m[f"w{i}"] = wp
        in_maps.append(m)
    return in_maps


def unshard_out(results):
    out = np.empty((ROWS_TOTAL, D), dtype=np.float32)
    for c, r in enumerate(results):
        buf = np.asarray(r["out"])  # flat [RPC * D] bf16, block+partition-major
        rows = slice(c * RPC, (c + 1) * RPC)
        for b, s in enumerate(SIZES):
            seg = buf[int(OFFS[b]) * RPC:int(OFFS[b + 1]) * RPC]
            # p-major: [p, r, e] -> rows r*128+p
            seg = seg.reshape(P, R_TILES, s).transpose(1, 0, 2).reshape(RPC, s)
            out[rows, int(OFFS[b]):int(OFFS[b + 1])] = seg
    return out


def kernel(x, w0, w1, w2, w3, w4, w5, w6, w7):
    nc = build_nc()
    in_maps = build_in_maps(x, w0, w1, w2, w3, w4, w5, w6, w7)
    res = run_bass_kernel_spmd(nc, in_maps, list(range(N_CORES)))
    return unshard_out(res.results).reshape(4, 2048, D)
